# revision 1
# baseline (speedup 1.0000x reference)
r"""DetCon (NT-Xent style) contrastive loss on 8 Trainium2 NeuronCores.

Reference computes, for v0/v1 = L2-normalized (over E) views scaled by
1/sqrt(T):   logits = [[S01, S00\diag], [S10, S11\diag]]  (2BN x 2BN-1)
             loss = mean_i( logsumexp(row_i) - label_logit_i )
with label_logit_i = S01[i,i] (== S10[i,i]).

Per-core plan (data-parallel over rows, host np.roll makes the program
core-independent):
  - load both views in natural [E, B*N] layout (2 x [128, 4096] f32 halves)
  - squares (gpsimd) -> column sumsq via ones-matmul (PE) -> per-column
    scale = exp(-0.5*ln(sumsq)) * sqrt(10) (ACT) -> partition-broadcast via
    K=1 matmul (PE) -> scale+downcast to bf16 (DVE)
  - 256 bf16 matmuls [128,512] (K=256) -> PSUM [128,2048] tiles
  - ACT exp with accum_out = fused row-sums; DVE mult+reduce with identity
    extracts label/diag values from PSUM
  - rowsum -= exp(diag_same_view)  (exact removal of the j==i term)
  - nll = ln(rowsum) - label; partition-reduce via ones-matmul -> scalar
Host sums the 8 per-core partial sums and divides by 2*B*N.
"""

import math
from contextlib import ExitStack

import numpy as np

import concourse.bacc as bacc
import concourse.bass as bass
import concourse.tile as tile
from concourse import mybir
from concourse.bass_utils import run_bass_kernel_spmd

B, E, N = 64, 256, 64
BN = B * N            # 4096 rows per view
NCORES = 8
CHUNK = BN // NCORES  # 512 rows (of each view) per core
P = 128
KH = E // P           # 2 contraction halves
G = 2048              # column group width (PSUM tile free dim)
NG = BN // G          # 2 column groups
TEMP = 0.1
# exp(-0.5*ln(s) + BIAS) = sqrt(10)/sqrt(s)
SCALE_BIAS = -0.5 * math.log(TEMP)

F32 = mybir.dt.float32
BF16 = mybir.dt.bfloat16


def _emit_pass(nc, pl, vin, out_dram, r, do_setup=True, do_main=True,
               nrm_prev=None):
    """Emit one full loss computation (rep r, for timing replication)."""
    ident, ones_col, ones_row, sbias = pl["consts"]

    if not do_setup:
        nrm = nrm_prev
    else:
        nrm = _emit_setup(nc, pl, vin, r)
    if not do_main:
        return nrm

    _emit_main(nc, pl, out_dram, r, nrm, ident, ones_col)
    return nrm


def _emit_setup(nc, pl, vin, r):
    ident, ones_col, ones_row, sbias = pl["consts"]
    # ---- load raw views in [E, B*N] layout (two 128-partition halves),
    # split per column-group across both HWDGE engines ----
    raw = [[None] * KH for _ in range(2)]
    GB = B // NG  # b-range per column group
    for v in range(2):
        for h in range(KH):
            t = pl["raw"].tile([P, BN], F32, tag=f"raw{v}{h}",
                               name=f"raw{v}{h}_{r}")
            for g in range(NG):
                src = vin[v][g * GB:(g + 1) * GB, h * P:(h + 1) * P, :] \
                    .rearrange("b e n -> e b n")
                dst = t[:, g * G:(g + 1) * G].rearrange(
                    "e (b n) -> e b n", b=GB)
                eng = nc.sync if (v + h) % 2 == 0 else nc.scalar
                eng.dma_start(out=dst, in_=src)
            raw[v][h] = t

    # ---- normalize: per-column scale, apply + downcast to bf16 ----
    nrm = [[pl["nrm"].tile([P, BN], BF16, tag=f"nrm{v}{h}",
                           name=f"nrm{v}{h}_{r}")
            for h in range(KH)] for v in range(2)]
    for g in range(NG):
        for v in range(2):
            gs = slice(g * G, (g + 1) * G)
            sq = [pl["sq"].tile([P, G], F32, tag="sq", name=f"sq{v}{g}{h}_{r}")
                  for h in range(KH)]
            for h in range(KH):
                nc.gpsimd.tensor_mul(
                    sq[h][:], raw[v][h][:, gs], raw[v][h][:, gs])
            ss = pl["psum"].tile([P, G], F32, tag="ps", name=f"ss{v}{g}_{r}")
            for j in range(G // 512):
                js = slice(j * 512, (j + 1) * 512)
                for h in range(KH):
                    nc.tensor.matmul(
                        ss[0:1, js], ones_col[:], sq[h][:, js],
                        start=(h == 0), stop=(h == KH - 1))
            lnb = pl["vec"].tile([1, G], F32, tag="lnb", name=f"lnb{v}{g}_{r}")
            nc.scalar.activation(
                lnb[:], ss[0:1, :], mybir.ActivationFunctionType.Ln)
            scl = pl["scl"].tile([1, G], F32, tag="scl", name=f"scl{v}{g}_{r}")
            nc.scalar.activation(
                scl[:], lnb[:], mybir.ActivationFunctionType.Exp,
                scale=-0.5, bias=sbias[:])
            pb = pl["psum"].tile([P, G], F32, tag="ps", name=f"pb{v}{g}_{r}")
            for j in range(G // 512):
                js = slice(j * 512, (j + 1) * 512)
                nc.tensor.matmul(pb[:, js], ones_row[:], scl[0:1, js])
            for h in range(KH):
                nc.vector.tensor_mul(nrm[v][h][:, gs], raw[v][h][:, gs], pb[:])
    return nrm


def _emit_main(nc, pl, out_dram, r, nrm, ident, ones_col):
    # per-pass collectors
    stats = pl["sml"].tile([P, 32], F32, tag="stats", name=f"stats{r}")
    diag01 = pl["sml"].tile([P, 8], F32, tag="diag01", name=f"diag01{r}")
    diag00 = pl["sml"].tile([P, 8], F32, tag="diag00", name=f"diag00{r}")
    # ---- main: logits row-blocks x column tiles, fused exp row-sums.
    # Column-group outer so group-0 logits overlap group-1 normalize. ----
    for g in range(NG):
        goff = g * G
        for half in range(2):       # 0: v0 rows, 1: v1 rows
            q = nrm[half]
            for m in range(4):      # 128-row blocks of this core's chunk
                hm = half * 4 + m
                ms = slice(m * P, (m + 1) * P)
                for tg in range(2):  # 0: cross-view keys, 1: same-view
                    keys = nrm[1 - half] if tg == 0 else nrm[half]
                    pt = pl["psum"].tile([P, G], F32, tag="ps",
                                         name=f"pt{g}{hm}{tg}_{r}")
                    for k in range(KH):
                        for j in range(G // 512):
                            js = slice(j * 512, (j + 1) * 512)
                            nc.tensor.matmul(
                                pt[:, js], q[k][:, ms],
                                keys[k][:, goff + j * 512:
                                        goff + (j + 1) * 512],
                                start=(k == 0), stop=(k == KH - 1))
                    if g == 0:
                        # tg==0: label logit (cross-view diag); tg==1:
                        # same-view diag (removed from row-sum later)
                        dst = diag01 if tg == 0 else diag00
                        dsc = pl["dsc"].tile([P, P], F32, tag="dsc",
                                             name=f"dsc{hm}{tg}_{r}")
                        nc.vector.tensor_mul(dsc[:], ident[:], pt[:, ms])
                        nc.vector.tensor_reduce(
                            dst[:, hm:hm + 1], dsc[:],
                            axis=mybir.AxisListType.X,
                            op=mybir.AluOpType.add)
                    esc = pl["esc"].tile([P, G], BF16, tag="esc",
                                         name=f"esc{g}{hm}{tg}_{r}")
                    sidx = hm * 4 + tg * 2 + g
                    nc.scalar.activation(
                        esc[:], pt[:, :], mybir.ActivationFunctionType.Exp,
                        accum_out=stats[:, sidx:sidx + 1])

    # ---- epilogue: nll partial sum ----
    ediag = pl["sml"].tile([P, 8], F32, tag="ediag", name=f"ediag{r}")
    nc.scalar.activation(ediag[:], diag00[:], mybir.ActivationFunctionType.Exp)
    rows = pl["sml"].tile([P, 8], F32, tag="rows", name=f"rows{r}")
    nc.vector.tensor_reduce(
        rows[:], stats[:].rearrange("p (m t) -> p m t", t=4),
        axis=mybir.AxisListType.X, op=mybir.AluOpType.add)
    nc.vector.tensor_sub(rows[:], rows[:], ediag[:])
    lnr = pl["sml"].tile([P, 8], F32, tag="lnr", name=f"lnr{r}")
    lnsum = pl["sml"].tile([P, 1], F32, tag="lnsum", name=f"lnsum{r}")
    nc.scalar.activation(
        lnr[:], rows[:], mybir.ActivationFunctionType.Ln, accum_out=lnsum[:])
    dsum = pl["sml"].tile([P, 1], F32, tag="dsum", name=f"dsum{r}")
    nc.vector.tensor_reduce(
        dsum[:], diag01[:], axis=mybir.AxisListType.X, op=mybir.AluOpType.add)
    tot = pl["sml"].tile([P, 1], F32, tag="tot", name=f"tot{r}")
    nc.vector.tensor_sub(tot[:], lnsum[:], dsum[:])
    fp = pl["psum"].tile([P, G], F32, tag="ps", name=f"fp{r}")
    nc.tensor.matmul(fp[0:1, 0:1], tot[:], ones_col[:])
    res = pl["sml"].tile([1, 1], F32, tag="res", name=f"res{r}")
    nc.vector.tensor_copy(res[:], fp[0:1, 0:1])
    nc.sync.dma_start(out=out_dram[:], in_=res[:])


def _build_nc(reps: int = 1, mode: str = "full"):
    """mode: 'full' reps everything; 'main' reps only the logits+exp phase
    (one shared setup); 'setup' reps only load+normalize."""
    nc = bacc.Bacc()
    vin = [
        nc.dram_tensor("view0", [B, E, N], F32, kind="ExternalInput"),
        nc.dram_tensor("view1", [B, E, N], F32, kind="ExternalInput"),
    ]
    ident_in = nc.dram_tensor("ident", [P, P], F32, kind="ExternalInput")
    out_dram = nc.dram_tensor("out", [1, 1], F32, kind="ExternalOutput")

    with ExitStack() as ctx:
        tc = ctx.enter_context(tile.TileContext(nc))
        pl = {
            name: ctx.enter_context(tc.tile_pool(name=name, bufs=bufs))
            for name, bufs in (("raw", 1), ("sq", 2), ("nrm", 1), ("vec", 2),
                               ("scl", 2), ("esc", 2), ("dsc", 2), ("sml", 1))
        }
        pl["psum"] = ctx.enter_context(
            tc.tile_pool(name="psum", bufs=2, space="PSUM"))

        ident = pl["sml"].tile([P, P], F32, tag="ident", name="ident")
        nc.sync.dma_start(out=ident[:], in_=ident_in[:])
        ones_col = pl["sml"].tile([P, 1], F32, tag="ones_col", name="ones_col")
        nc.vector.memset(ones_col[:], 1.0)
        ones_row = pl["sml"].tile([1, P], F32, tag="ones_row", name="ones_row")
        nc.vector.memset(ones_row[:], 1.0)
        sbias = pl["sml"].tile([1, 1], F32, tag="sbias", name="sbias")
        nc.vector.memset(sbias[:], SCALE_BIAS)
        pl["consts"] = (ident, ones_col, ones_row, sbias)

        nrm = None
        for r in range(reps):
            nrm = _emit_pass(
                nc, pl, vin, out_dram, r,
                do_setup=(mode != "main" or r == 0),
                do_main=(mode != "setup"),
                nrm_prev=nrm)

    nc.compile()
    return nc


_NC_CACHE = None


def _run_spmd(view0: np.ndarray, view1: np.ndarray, nc=None, **spmd_kwargs):
    global _NC_CACHE
    if nc is None:
        if _NC_CACHE is None:
            _NC_CACHE = _build_nc()
        nc = _NC_CACHE

    ident = np.eye(P, dtype=np.float32)
    in_maps = []
    for c in range(NCORES):
        in_maps.append({
            "view0": np.ascontiguousarray(np.roll(view0, -c * (B // NCORES), axis=0)),
            "view1": np.ascontiguousarray(np.roll(view1, -c * (B // NCORES), axis=0)),
            "ident": ident,
        })
    res = run_bass_kernel_spmd(nc, in_maps, core_ids=list(range(NCORES)),
                               **spmd_kwargs)
    total = sum(float(r["out"][0, 0]) for r in res.results)
    return np.float32(total / (2 * BN)), res


def kernel(view0: np.ndarray, view1: np.ndarray) -> np.ndarray:
    loss, _ = _run_spmd(view0, view1)
    return loss



# revision 16
# speedup vs baseline: 1.1473x; 1.1473x over previous
r"""DetCon (NT-Xent style) contrastive loss on 8 Trainium2 NeuronCores.

Reference: v0/v1 L2-normalized (over E) scaled by 1/sqrt(T);
  logits = [[S01, S00\diag], [S10, S11\diag]]  (2BN x 2BN-1)
  loss = mean_i(logsumexp(row_i) - label_i),  label_i = S01[i,i].

Per-core plan (data-parallel rows; host np.roll makes the program
core-independent; each core's rows are cols 0..511 of each view):
  setup(r):  8x 1MB DMA raw [E, BN] f32 (sync queue); squares (DVE +
    gpsimd, bf16); per-column sumsq via ones-matmul (bf16, 1 cyc/row)
    -> [1,2048] PSUM; tiny reshape-DMA -> [8,512] SBUF; ACT Ln+Exp
    (one table set) -> scl; one-hot-row broadcast matmuls (f32r) ->
    pb [128,2048] PSUM; DVE raw*pb -> bf16 nrm.
  main(r):   per 128-row block x 2048-key tile: 8 bf16 matmuls K=256
    -> PSUM; ACT exp with accum_out row-sums (32 tiles = the ACT
    bottleneck, ~2.04us each).
  epilogue:  row-sums - exp(10) (same-view diag is exactly 10, so no
    diag extraction); ln via ACT with bias=-exp(10), accum_out;
    partition-reduce via ones-matmul; labels = colsum(nrm0*nrm1).
Reps are software-pipelined: setup(r) emission is interleaved into
main(r-1) so every engine queue stays busy; steady state is ACT-bound.
Host sums the 8 per-core partials and divides by 2*B*N.
"""

import math
from contextlib import ExitStack

import numpy as np

import concourse.bacc as bacc
import concourse.bass as bass
import concourse.tile as tile
from concourse import mybir
from concourse.bass_utils import run_bass_kernel_spmd

B, E, N = 64, 256, 64
BN = B * N            # 4096 rows per view
NCORES = 8
P = 128
KH = E // P           # 2 contraction halves
G = 2048              # column group width (PSUM tile free dim)
NG = BN // G          # 2 column groups
GB = B // NG          # b-range per column group
TEMP = 0.1
SCALE_BIAS = -0.5 * math.log(TEMP)   # exp(-0.5*ln(s) + BIAS) = sqrt(10/s)
EXP10 = float(np.exp(np.float64(10.0)))  # exact same-view diag: |q|^2 = 10

F32 = mybir.dt.float32
F32R = mybir.dt.float32r
BF16 = mybir.dt.bfloat16
AFT = mybir.ActivationFunctionType


def _main_tile_list():
    """(g, half, m, tg) in emission order: 16 g0 tiles then 16 g1 tiles."""
    out = []
    for g in range(NG):
        for half in range(2):
            for m in range(4):
                for tg in range(2):
                    out.append((g, half, m, tg))
    return out


class _Emitter:
    def __init__(self, nc, pl):
        self.nc = nc
        self.pl = pl
        self.ones_col = None
        self.ones_col_b = None
        self.onesel = None
        # per-rep state
        self.raw = {}     # r -> [v][h] tiles
        self.sq = {}      # (r, g) -> {(v, h): tile}
        self.nrm = {}     # r -> [v][h] tiles
        self.scl = {}     # (r, g) -> scl16 tile
        self.stats = {}   # r -> stats tile
        self.lbl2 = {}    # r -> 2*sum(labels) tile

    def emit_consts(self):
        nc, pl = self.nc, self.pl
        self.ones_col = pl["cst"].tile([P, 1], F32, tag="ones_col",
                                       name="ones_col")
        nc.vector.memset(self.ones_col[:], 1.0)
        self.ones_col_b = pl["cst"].tile([P, 1], BF16, tag="ones_col_b",
                                         name="ones_col_b")
        nc.vector.memset(self.ones_col_b[:], 1.0)
        self.ones_row = pl["cst"].tile([1, P], BF16, tag="ones_row",
                                       name="ones_row")
        nc.vector.memset(self.ones_row[:], 1.0)
        self.sbias = pl["cst"].tile([8, 1], F32, tag="sbias", name="sbias")
        nc.vector.memset(self.sbias[:], SCALE_BIAS)
        self.nexp10 = pl["cst"].tile([P, 1], F32, tag="nexp10", name="nexp10")
        nc.vector.memset(self.nexp10[:], -EXP10)
        self.zbias = pl["cst"].tile([8, 1], F32, tag="zbias", name="zbias")
        nc.vector.memset(self.zbias[:], 0.0)

    # ---- setup pieces -------------------------------------------------
    def setup_dma_and_sq_g0(self, r, vin):
        """Raw loads for the whole rep + squares for g0 (and gpsimd g1)."""
        nc, pl = self.nc, self.pl
        raw = [[pl["raw"].tile([P, BN], F32, tag=f"raw{v}{h}",
                               name=f"raw{v}{h}_{r}")
                for h in range(KH)] for v in range(2)]
        self.raw[r] = raw
        for g in range(NG):
            for v in range(2):
                for h in range(KH):
                    src = vin[v][g * GB:(g + 1) * GB, h * P:(h + 1) * P, :] \
                        .rearrange("b e n -> e b n")
                    dst = raw[v][h][:, g * G:(g + 1) * G].rearrange(
                        "e (b n) -> e b n", b=GB)
                    nc.sync.dma_start(out=dst, in_=src)
        self.nrm[r] = [[pl["nrm"].tile([P, BN], BF16, tag=f"nrm{v}{h}",
                                       name=f"nrm{v}{h}_{r}")
                        for h in range(KH)] for v in range(2)]
        # squares: h==0 on DVE, h==1 on gpsimd; gpsimd also takes g1 now
        self._emit_sq(r, 0, engines=("vector", "gpsimd"))
        self._emit_sq(r, 1, engines=(None, "gpsimd"))

    def _emit_sq(self, r, g, engines):
        nc, pl = self.nc, self.pl
        d = self.sq.setdefault((r, g), {})
        gs = slice(g * G, (g + 1) * G)
        for v in range(2):
            for h in range(KH):
                eng = engines[h]
                if eng is None or (v, h) in d:
                    continue
                t = pl["sq"].tile([P, G], BF16, tag="sq", name=f"sq{v}{h}{g}_{r}")
                getattr(nc, eng).tensor_mul(
                    t[:], self.raw[r][v][h][:, gs], self.raw[r][v][h][:, gs])
                d[(v, h)] = t

    def setup_finish_g(self, r, g):
        """Colsum -> reshape -> Ln/Exp -> broadcast -> apply for group g.
        For g==0 also emits the DVE squares of g1 first (data ready)."""
        nc, pl = self.nc, self.pl
        if g == 0:
            self._emit_sq(r, 1, engines=("vector", None))
        sq = self.sq[(r, g)]
        gs = slice(g * G, (g + 1) * G)
        # per-column sumsq -> ss[0:1, :] (4 blocks of 512 per view)
        sss = []
        for v in range(2):
            ss = pl["psum"].tile([P, G], F32, tag="ps", name=f"ss{v}{g}_{r}")
            for b in range(4):
                js = slice(b * 512, (b + 1) * 512)
                for h in range(KH):
                    nc.tensor.matmul(
                        ss[0:1, js], self.ones_col_b[:], sq[(v, h)][:, js],
                        start=(h == 0), stop=(h == KH - 1))
            sss.append(ss)
        # PSUM -> SBUF bounce (DMA can't read PSUM), reshape [1,2048] ->
        # [4,512] rows of the [8,512] stage so Ln/Exp use 8 ACT lanes,
        # then reshape back to [1,2048] rows for the K=1 broadcast matmul.
        sstg = pl["stg"].tile([8, 512], F32, tag="sstg", name=f"sstg{g}_{r}")
        for v in range(2):
            sres = pl["stg"].tile([1, G], F32, tag="sres",
                                  name=f"sres{v}{g}_{r}")
            nc.vector.tensor_copy(sres[:], sss[v][0:1, :])
            nc.sync.dma_start(out=sstg[v * 4:(v + 1) * 4, :], in_=sres[:])
        lnstg = pl["stg"].tile([8, 512], F32, tag="lnstg", name=f"ln{g}_{r}")
        nc.scalar.activation(lnstg[:], sstg[:], AFT.Ln, bias=self.zbias[:])
        scl16 = pl["stg"].tile([8, 512], BF16, tag="scl16", name=f"scl{g}_{r}")
        nc.scalar.activation(scl16[:], lnstg[:], AFT.Exp,
                             scale=-0.5, bias=self.sbias[:])
        self.scl[(r, g)] = scl16
        # broadcast scale rows to pb tiles, then apply to raw -> nrm (bf16)
        for v in range(2):
            srow = pl["stg"].tile([1, G], BF16, tag="srow",
                                  name=f"srow{v}{g}_{r}")
            nc.sync.dma_start(out=srow[:], in_=scl16[v * 4:(v + 1) * 4, :])
            pb = pl["psum"].tile([P, G], F32, tag="ps", name=f"pb{v}{g}_{r}")
            for b in range(4):
                js = slice(b * 512, (b + 1) * 512)
                nc.tensor.matmul(
                    pb[:, js], self.ones_row[:], srow[0:1, js])
            for h in range(KH):
                nc.vector.tensor_mul(
                    self.nrm[r][v][h][:, gs], self.raw[r][v][h][:, gs], pb[:])

    def emit_label(self, r):
        """2 * sum_i(label_i) for this core's 512 rows (bf16 path)."""
        nc, pl = self.nc, self.pl
        nrm = self.nrm[r]
        tmps = []
        for h in range(KH):
            t = pl["sml"].tile([P, 512], BF16, tag=f"lblt{h}",
                               name=f"lblt{h}_{r}")
            nc.vector.tensor_mul(t[:], nrm[0][h][:, 0:512],
                                 nrm[1][h][:, 0:512])
            tmps.append(t)
        lbl = pl["psum"].tile([P, G], F32, tag="ps", name=f"lbl_{r}")
        for h in range(KH):
            nc.tensor.matmul(lbl[0:1, 0:512], self.ones_col_b[:], tmps[h][:],
                             start=(h == 0), stop=(h == KH - 1))
        lbls = pl["sml"].tile([1, 1], F32, tag="lbls", name=f"lbls_{r}")
        nc.vector.tensor_reduce(lbls[:], lbl[0:1, 0:512],
                                axis=mybir.AxisListType.X,
                                op=mybir.AluOpType.add)
        lbl2 = pl["sml"].tile([1, 1], F32, tag="lbl2", name=f"lbl2_{r}")
        nc.vector.tensor_scalar_mul(lbl2[:], lbls[:], 2.0)
        self.lbl2[r] = lbl2

    # ---- main pieces --------------------------------------------------
    def main_tiles(self, r, tiles):
        nc, pl = self.nc, self.pl
        nrm = self.nrm[r]
        if r not in self.stats:
            self.stats[r] = pl["sml"].tile([P, 32], F32, tag="stats",
                                           name=f"stats_{r}")
        stats = self.stats[r]
        for (g, half, m, tg) in tiles:
            goff = g * G
            ms = slice(m * P, (m + 1) * P)
            keys = nrm[1 - half] if tg == 0 else nrm[half]
            pt = pl["psum"].tile([P, G], F32, tag="ps",
                                 name=f"pt{g}{half}{m}{tg}_{r}")
            for k in range(KH):
                for j in range(4):
                    js = slice(j * 512, (j + 1) * 512)
                    nc.tensor.matmul(
                        pt[:, js], nrm[half][k][:, ms],
                        keys[k][:, goff + j * 512: goff + (j + 1) * 512],
                        start=(k == 0), stop=(k == KH - 1))
            esc = pl["esc"].tile([P, G], BF16, tag="esc",
                                 name=f"esc{g}{half}{m}{tg}_{r}")
            sidx = (half * 4 + m) * 4 + tg * 2 + g
            nc.scalar.activation(esc[:], pt[:, :], AFT.Exp,
                                 accum_out=stats[:, sidx:sidx + 1])

    def emit_epilogue(self, r, out_dram):
        nc, pl = self.nc, self.pl
        stats = self.stats[r]
        rows = pl["sml"].tile([P, 8], F32, tag="rows", name=f"rows_{r}")
        nc.vector.tensor_reduce(
            rows[:], stats[:].rearrange("p (m t) -> p m t", t=4),
            axis=mybir.AxisListType.X, op=mybir.AluOpType.add)
        lnr = pl["sml"].tile([P, 8], F32, tag="lnr", name=f"lnr_{r}")
        lnsum = pl["sml"].tile([P, 1], F32, tag="lnsum", name=f"lnsum_{r}")
        # ln(rowsum - exp(10)): removes the same-view diag term exactly
        nc.scalar.activation(lnr[:], rows[:], AFT.Ln, bias=self.nexp10[:],
                             accum_out=lnsum[:])
        fp = pl["psum"].tile([P, G], F32, tag="ps", name=f"fp_{r}")
        nc.tensor.matmul(fp[0:1, 0:1], lnsum[:], self.ones_col[:])
        res = pl["sml"].tile([1, 1], F32, tag="res", name=f"res_{r}")
        nc.vector.tensor_sub(res[:], fp[0:1, 0:1], self.lbl2[r][:])
        nc.sync.dma_start(out=out_dram[:], in_=res[:])
        # free per-rep references
        del self.raw[r], self.nrm[r], self.stats[r], self.lbl2[r]
        self.sq.pop((r, 0), None)
        self.sq.pop((r, 1), None)
        self.scl.pop((r, 0), None)
        self.scl.pop((r, 1), None)


def _build_nc(reps: int = 1, mode: str = "full"):
    nc = bacc.Bacc()
    vin = [
        nc.dram_tensor("view0", [B, E, N], F32, kind="ExternalInput"),
        nc.dram_tensor("view1", [B, E, N], F32, kind="ExternalInput"),
    ]
    out_dram = nc.dram_tensor("out", [1, 1], F32, kind="ExternalOutput")

    with ExitStack() as ctx:
        tc = ctx.enter_context(tile.TileContext(nc))
        pl = {
            "raw": ctx.enter_context(tc.tile_pool(name="raw", bufs=1)),
            "nrm": ctx.enter_context(tc.tile_pool(name="nrm", bufs=2)),
            "sq": ctx.enter_context(tc.tile_pool(name="sq", bufs=6)),
            "esc": ctx.enter_context(tc.tile_pool(name="esc", bufs=2)),
            "stg": ctx.enter_context(tc.tile_pool(name="stg", bufs=2)),
            "sml": ctx.enter_context(tc.tile_pool(name="sml", bufs=2)),
            "cst": ctx.enter_context(tc.tile_pool(name="cst", bufs=1)),
            "psum": ctx.enter_context(
                tc.tile_pool(name="psum", bufs=2, space="PSUM")),
        }
        em = _Emitter(nc, pl)
        em.emit_consts()
        tiles = _main_tile_list()
        for r in range(reps):
            em.setup_dma_and_sq_g0(r, vin)
            if r > 0:
                em.main_tiles(r - 1, tiles[0:18])
            em.setup_finish_g(r, 0)
            if r > 0:
                em.main_tiles(r - 1, tiles[18:26])
            em.setup_finish_g(r, 1)
            if r > 0:
                em.main_tiles(r - 1, tiles[26:32])
            em.emit_label(r)
            if r > 0:
                em.emit_epilogue(r - 1, out_dram)
        em.main_tiles(reps - 1, tiles)
        em.emit_epilogue(reps - 1, out_dram)

    nc.compile()
    return nc


_NC_CACHE = None


def _run_spmd(view0: np.ndarray, view1: np.ndarray, nc=None, **spmd_kwargs):
    global _NC_CACHE
    if nc is None:
        if _NC_CACHE is None:
            _NC_CACHE = _build_nc()
        nc = _NC_CACHE

    in_maps = []
    for c in range(NCORES):
        in_maps.append({
            "view0": np.ascontiguousarray(
                np.roll(view0, -c * (B // NCORES), axis=0)),
            "view1": np.ascontiguousarray(
                np.roll(view1, -c * (B // NCORES), axis=0)),
        })
    res = run_bass_kernel_spmd(nc, in_maps, core_ids=list(range(NCORES)),
                               **spmd_kwargs)
    total = sum(float(r["out"][0, 0]) for r in res.results)
    return np.float32(total / (2 * BN)), res


def kernel(view0: np.ndarray, view1: np.ndarray) -> np.ndarray:
    loss, _ = _run_spmd(view0, view1)
    return loss


# revision 28
# speedup vs baseline: 1.4644x; 1.2764x over previous
r"""DetCon (NT-Xent style) contrastive loss on 8 Trainium2 NeuronCores.

Reference: v0/v1 L2-normalized (over E) scaled by 1/sqrt(T);
  logits = [[S01, S00\diag], [S10, S11\diag]]  (2BN x 2BN-1)
  loss = mean_i(logsumexp(row_i) - label_i),  label_i = S01[i,i].

Per-core plan (data-parallel rows; host np.roll makes the program
core-independent; each core's rows are cols 0..511 of each view):
  setup(r):  8x 1MB DMA raw [E, BN] f32 (sync queue); squares (DVE +
    gpsimd, bf16); per-column sumsq via ones-matmul (bf16, 1 cyc/row)
    -> [1,2048] PSUM; tiny reshape-DMA -> [8,512] SBUF; ACT Ln+Exp
    (one table set) -> scl; one-hot-row broadcast matmuls (f32r) ->
    pb [128,2048] PSUM; DVE raw*pb -> bf16 nrm.
  main(r):   per 128-row block x 2048-key tile: 8 bf16 matmuls K=256
    -> PSUM; ACT exp with accum_out row-sums (32 tiles = the ACT
    bottleneck, ~2.04us each).
  epilogue:  row-sums - exp(10) (same-view diag is exactly 10, so no
    diag extraction); ln via ACT with bias=-exp(10), accum_out;
    partition-reduce via ones-matmul; labels = colsum(nrm0*nrm1).
Reps are software-pipelined: setup(r) emission is interleaved into
main(r-1) so every engine queue stays busy; steady state is ACT-bound.
Host sums the 8 per-core partials and divides by 2*B*N.
"""

import math
from contextlib import ExitStack

import numpy as np

import concourse.bacc as bacc
import concourse.bass as bass
import concourse.tile as tile
from concourse import mybir
from concourse.bass_utils import run_bass_kernel_spmd

B, E, N = 64, 256, 64
BN = B * N            # 4096 rows per view
NCORES = 8
P = 128
KH = E // P           # 2 contraction halves
G = 2048              # column group width (PSUM tile free dim)
NG = BN // G          # 2 column groups
GB = B // NG          # b-range per column group
TEMP = 0.1
SCALE_BIAS = -0.5 * math.log(TEMP)   # exp(-0.5*ln(s) + BIAS) = sqrt(10/s)
EXP10 = float(np.exp(np.float64(10.0)))  # exact same-view diag: |q|^2 = 10

F32 = mybir.dt.float32
F32R = mybir.dt.float32r
BF16 = mybir.dt.bfloat16
AFT = mybir.ActivationFunctionType

# We alternate Ln and Exp on the ACT engine every rep. The table-load
# inserter picks the first set containing each function, which puts Ln and
# Exp in different sets and forces a ~1.3us ACT table reload per switch.
# Hide Exp/Ln from every set except the one that contains both, so all
# activations share one resident table (json set indices are preserved).
_orig_gat = bacc.get_activation_tables


def _gat_ln_exp_combined(arch):
    tabs = {k: set(v) for k, v in _orig_gat(arch).items()}
    for name, s in tabs.items():
        if name != "natural_log_exp_and_others":
            s.discard(AFT.Exp)
            s.discard(AFT.Ln)
    return tabs


bacc.get_activation_tables = _gat_ln_exp_combined


def _main_tile_list():
    """(g, half, m, tg) in emission order: 16 g0 tiles then 16 g1 tiles."""
    out = []
    for g in range(NG):
        for half in range(2):
            for m in range(4):
                for tg in range(2):
                    out.append((g, half, m, tg))
    return out


class _Emitter:
    def __init__(self, nc, pl):
        self.nc = nc
        self.pl = pl
        self.ones_col = None
        self.ones_col_b = None
        self.onesel = None
        # per-rep state
        self.raw = {}     # r -> [v][h] tiles
        self.sq = {}      # (r, g) -> {(v, h): tile}
        self.nrm = {}     # r -> [v][h] tiles
        self.scl = {}     # (r, g) -> scl16 tile
        self.stats = {}   # r -> stats tile
        self.lbl2 = {}    # r -> 2*sum(labels) tile

    def emit_consts(self):
        nc, pl = self.nc, self.pl
        self.ones_col = pl["cst"].tile([P, 1], F32, tag="ones_col",
                                       name="ones_col")
        nc.vector.memset(self.ones_col[:], 1.0)
        self.ones_col_b = pl["cst"].tile([P, 1], BF16, tag="ones_col_b",
                                         name="ones_col_b")
        nc.vector.memset(self.ones_col_b[:], 1.0)
        self.ones_row = pl["cst"].tile([1, P], BF16, tag="ones_row",
                                       name="ones_row")
        nc.vector.memset(self.ones_row[:], 1.0)
        self.sbias = pl["cst"].tile([8, 1], F32, tag="sbias", name="sbias")
        nc.vector.memset(self.sbias[:], SCALE_BIAS)
        self.nexp10 = pl["cst"].tile([P, 1], F32, tag="nexp10", name="nexp10")
        nc.vector.memset(self.nexp10[:], -EXP10)
        self.zbias = pl["cst"].tile([8, 1], F32, tag="zbias", name="zbias")
        nc.vector.memset(self.zbias[:], 0.0)

    # ---- setup pieces -------------------------------------------------
    def setup_dma_and_sq_g0(self, r, vin):
        """Raw loads for the whole rep + squares for g0 (and gpsimd g1)."""
        nc, pl = self.nc, self.pl
        raw = [[pl["raw"].tile([P, BN], F32, tag=f"raw{v}{h}",
                               name=f"raw{v}{h}_{r}")
                for h in range(KH)] for v in range(2)]
        self.raw[r] = raw
        for g in range(NG):
            for v in range(2):
                for h in range(KH):
                    src = vin[v][g * GB:(g + 1) * GB, h * P:(h + 1) * P, :] \
                        .rearrange("b e n -> e b n")
                    dst = raw[v][h][:, g * G:(g + 1) * G].rearrange(
                        "e (b n) -> e b n", b=GB)
                    eng = nc.sync if (v + h) % 2 == 0 else nc.scalar
                    eng.dma_start(out=dst, in_=src)
        self.nrm[r] = [[pl["nrm"].tile([P, BN], BF16, tag=f"nrm{v}{h}",
                                       name=f"nrm{v}{h}_{r}")
                        for h in range(KH)] for v in range(2)]
        # squares: h==0 on DVE, h==1 on gpsimd; gpsimd also takes g1 now
        self._emit_sq(r, 0, engines=("vector", "gpsimd"))
        self._emit_sq(r, 1, engines=(None, "gpsimd"))

    def _emit_sq(self, r, g, engines):
        nc, pl = self.nc, self.pl
        d = self.sq.setdefault((r, g), {})
        gs = slice(g * G, (g + 1) * G)
        for v in range(2):
            for h in range(KH):
                eng = engines[h]
                if eng is None or (v, h) in d:
                    continue
                t = pl["sq"].tile([P, G], BF16, tag="sq", name=f"sq{v}{h}{g}_{r}")
                getattr(nc, eng).tensor_mul(
                    t[:], self.raw[r][v][h][:, gs], self.raw[r][v][h][:, gs])
                d[(v, h)] = t

    def setup_colsum_g(self, r, g):
        """Per-column sumsq matmuls -> PSUM row -> SBUF [8,512] stage."""
        nc, pl = self.nc, self.pl
        if g == 0:
            self._emit_sq(r, 1, engines=("vector", None))
        sq = self.sq[(r, g)]
        sstg = pl["stg"].tile([8, 512], F32, tag="sstg", name=f"sstg{g}_{r}")
        for v in range(2):
            ss = pl["psum"].tile([P, G], F32, tag="ps", name=f"ss{v}{g}_{r}")
            for b in range(4):
                js = slice(b * 512, (b + 1) * 512)
                for h in range(KH):
                    nc.tensor.matmul(
                        ss[0:1, js], self.ones_col_b[:], sq[(v, h)][:, js],
                        start=(h == 0), stop=(h == KH - 1))
            # PSUM -> SBUF bounce (DMA can't read PSUM), reshape [1,2048]
            # -> [4,512] rows so Ln/Exp use 8 ACT lanes
            sres = pl["stg"].tile([1, G], F32, tag="sres",
                                  name=f"sres{v}{g}_{r}")
            nc.vector.tensor_copy(sres[:], ss[0:1, :])
            nc.gpsimd.dma_start(out=sstg[v * 4:(v + 1) * 4, :], in_=sres[:])
        self.scl[(r, g, "sstg")] = sstg

    def setup_scale_g(self, r, g):
        """Ln/Exp -> stride-0 broadcast DMA into SBUF pb -> apply."""
        nc, pl = self.nc, self.pl
        sstg = self.scl.pop((r, g, "sstg"))
        gs = slice(g * G, (g + 1) * G)
        lnstg = pl["stg"].tile([8, 512], F32, tag="lnstg", name=f"ln{g}_{r}")
        nc.scalar.activation(lnstg[:], sstg[:], AFT.Ln, bias=self.zbias[:])
        scl16 = pl["stg"].tile([8, 512], BF16, tag="scl16", name=f"scl{g}_{r}")
        nc.scalar.activation(scl16[:], lnstg[:], AFT.Exp,
                             scale=-0.5, bias=self.sbias[:])
        self.scl[(r, g)] = scl16
        # broadcast each scale row across 128 partitions via DMA (keeps the
        # PE free of any dependency on the ACT-produced scales)
        for v in range(2):
            srow = pl["stg"].tile([1, G], BF16, tag="srow",
                                  name=f"srow{v}{g}_{r}")
            nc.gpsimd.dma_start(out=srow[:], in_=scl16[v * 4:(v + 1) * 4, :])
            pb = pl["pbs"].tile([P, G], BF16, tag=f"pb{v}",
                                name=f"pb{v}{g}_{r}")
            nc.gpsimd.partition_broadcast(pb[:], srow[0:1, :])
            for h in range(KH):
                nc.vector.tensor_mul(
                    self.nrm[r][v][h][:, gs], self.raw[r][v][h][:, gs], pb[:])

    def emit_label(self, r):
        """2 * sum_i(label_i) for this core's 512 rows (bf16 path)."""
        nc, pl = self.nc, self.pl
        nrm = self.nrm[r]
        tmps = []
        for h in range(KH):
            t = pl["sml"].tile([P, 512], BF16, tag=f"lblt{h}",
                               name=f"lblt{h}_{r}")
            nc.vector.tensor_mul(t[:], nrm[0][h][:, 0:512],
                                 nrm[1][h][:, 0:512])
            tmps.append(t)
        lbl = pl["psum"].tile([P, G], F32, tag="ps", name=f"lbl_{r}")
        for h in range(KH):
            nc.tensor.matmul(lbl[0:1, 0:512], self.ones_col_b[:], tmps[h][:],
                             start=(h == 0), stop=(h == KH - 1))
        lbls = pl["sml"].tile([1, 1], F32, tag="lbls", name=f"lbls_{r}")
        nc.vector.tensor_reduce(lbls[:], lbl[0:1, 0:512],
                                axis=mybir.AxisListType.X,
                                op=mybir.AluOpType.add)
        lbl2 = pl["sml"].tile([1, 1], F32, tag="lbl2", name=f"lbl2_{r}")
        nc.vector.tensor_scalar_mul(lbl2[:], lbls[:], 2.0)
        self.lbl2[r] = lbl2

    # ---- main pieces --------------------------------------------------
    def main_tiles(self, r, tiles):
        nc, pl = self.nc, self.pl
        nrm = self.nrm[r]
        if r not in self.stats:
            self.stats[r] = pl["sml"].tile([P, 32], F32, tag="stats",
                                           name=f"stats_{r}")
        stats = self.stats[r]
        for (g, half, m, tg) in tiles:
            goff = g * G
            ms = slice(m * P, (m + 1) * P)
            keys = nrm[1 - half] if tg == 0 else nrm[half]
            pt = pl["psum"].tile([P, G], F32, tag="ps",
                                 name=f"pt{g}{half}{m}{tg}_{r}")
            for k in range(KH):
                for j in range(4):
                    js = slice(j * 512, (j + 1) * 512)
                    nc.tensor.matmul(
                        pt[:, js], nrm[half][k][:, ms],
                        keys[k][:, goff + j * 512: goff + (j + 1) * 512],
                        start=(k == 0), stop=(k == KH - 1))
            esc = pl["esc"].tile([P, G], BF16, tag="esc",
                                 name=f"esc{g}{half}{m}{tg}_{r}")
            sidx = (half * 4 + m) * 4 + tg * 2 + g
            nc.scalar.activation(esc[:], pt[:, :], AFT.Exp,
                                 accum_out=stats[:, sidx:sidx + 1])

    def emit_epilogue(self, r, out_dram):
        nc, pl = self.nc, self.pl
        stats = self.stats[r]
        rows = pl["sml"].tile([P, 8], F32, tag="rows", name=f"rows_{r}")
        nc.vector.tensor_reduce(
            rows[:], stats[:].rearrange("p (m t) -> p m t", t=4),
            axis=mybir.AxisListType.X, op=mybir.AluOpType.add)
        lnr = pl["sml"].tile([P, 8], F32, tag="lnr", name=f"lnr_{r}")
        lnsum = pl["sml"].tile([P, 1], F32, tag="lnsum", name=f"lnsum_{r}")
        # ln(rowsum - exp(10)): removes the same-view diag term exactly
        nc.scalar.activation(lnr[:], rows[:], AFT.Ln, bias=self.nexp10[:],
                             accum_out=lnsum[:])
        fp = pl["psum"].tile([P, G], F32, tag="ps", name=f"fp_{r}")
        nc.tensor.matmul(fp[0:1, 0:1], lnsum[:], self.ones_col[:])
        res = pl["sml"].tile([1, 1], F32, tag="res", name=f"res_{r}")
        nc.vector.tensor_sub(res[:], fp[0:1, 0:1], self.lbl2[r][:])
        nc.sync.dma_start(out=out_dram[:], in_=res[:])
        # free per-rep references
        for d in (self.raw, self.nrm, self.stats, self.lbl2):
            d.pop(r, None)
        for g in range(NG):
            self.sq.pop((r, g), None)
            self.scl.pop((r, g), None)


def _build_nc(reps: int = 1, mode: str = "full"):
    nc = bacc.Bacc()
    vin = [
        nc.dram_tensor("view0", [B, E, N], F32, kind="ExternalInput"),
        nc.dram_tensor("view1", [B, E, N], F32, kind="ExternalInput"),
    ]
    out_dram = nc.dram_tensor("out", [1, 1], F32, kind="ExternalOutput")

    with ExitStack() as ctx:
        tc = ctx.enter_context(tile.TileContext(nc))
        pl = {
            "raw": ctx.enter_context(tc.tile_pool(name="raw", bufs=1)),
            "nrm": ctx.enter_context(tc.tile_pool(name="nrm", bufs=2)),
            "sq": ctx.enter_context(tc.tile_pool(name="sq", bufs=5)),
            "esc": ctx.enter_context(tc.tile_pool(name="esc", bufs=1)),
            "pbs": ctx.enter_context(tc.tile_pool(name="pbs", bufs=2)),
            "stg": ctx.enter_context(tc.tile_pool(name="stg", bufs=2)),
            "sml": ctx.enter_context(tc.tile_pool(name="sml", bufs=2)),
            "cst": ctx.enter_context(tc.tile_pool(name="cst", bufs=1)),
            "psum": ctx.enter_context(
                tc.tile_pool(name="psum", bufs=2, space="PSUM")),
        }
        em = _Emitter(nc, pl)
        em.emit_consts()
        tiles = _main_tile_list()
        if mode == "main":
            # one setup, then reps x main phase (timing attribution)
            em.setup_dma_and_sq_g0(0, vin)
            for g in range(NG):
                em.setup_colsum_g(0, g)
                em.setup_scale_g(0, g)
            em.emit_label(0)
            nrm0, lbl20 = em.nrm[0], em.lbl2[0]
            for r in range(reps):
                em.nrm[r], em.lbl2[r] = nrm0, lbl20
                em.main_tiles(r, tiles)
                em.emit_epilogue(r, out_dram)
                em.nrm[0], em.lbl2[0] = nrm0, lbl20
            nc.compile()
            return nc
        if mode == "setup":
            for r in range(reps):
                em.setup_dma_and_sq_g0(r, vin)
                for g in range(NG):
                    em.setup_colsum_g(r, g)
                    em.setup_scale_g(r, g)
                em.emit_label(r)
                del em.raw[r], em.nrm[r]
            nc.compile()
            return nc
        for r in range(reps):
            em.setup_dma_and_sq_g0(r, vin)
            if r > 0:
                em.main_tiles(r - 1, tiles[0:12])
            em.setup_colsum_g(r, 0)
            if r > 0:
                em.main_tiles(r - 1, tiles[12:16])
            em.setup_scale_g(r, 0)
            if r > 0:
                em.main_tiles(r - 1, tiles[16:24])
            em.setup_colsum_g(r, 1)
            if r > 0:
                em.main_tiles(r - 1, tiles[24:26])
            em.setup_scale_g(r, 1)
            if r > 0:
                em.main_tiles(r - 1, tiles[26:32])
            em.emit_label(r)
            if r > 0:
                em.emit_epilogue(r - 1, out_dram)
        em.main_tiles(reps - 1, tiles)
        em.emit_epilogue(reps - 1, out_dram)

    nc.compile()
    return nc


_NC_CACHE = None


def _run_spmd(view0: np.ndarray, view1: np.ndarray, nc=None, **spmd_kwargs):
    global _NC_CACHE
    if nc is None:
        if _NC_CACHE is None:
            _NC_CACHE = _build_nc()
        nc = _NC_CACHE

    in_maps = []
    for c in range(NCORES):
        in_maps.append({
            "view0": np.ascontiguousarray(
                np.roll(view0, -c * (B // NCORES), axis=0)),
            "view1": np.ascontiguousarray(
                np.roll(view1, -c * (B // NCORES), axis=0)),
        })
    res = run_bass_kernel_spmd(nc, in_maps, core_ids=list(range(NCORES)),
                               **spmd_kwargs)
    total = sum(float(r["out"][0, 0]) for r in res.results)
    return np.float32(total / (2 * BN)), res


def kernel(view0: np.ndarray, view1: np.ndarray) -> np.ndarray:
    loss, _ = _run_spmd(view0, view1)
    return loss


# revision 48
# speedup vs baseline: 17.0862x; 11.6681x over previous
r"""DetCon (NT-Xent style) contrastive loss on 8 Trainium2 NeuronCores.

Reference: v0/v1 L2-normalized (over E) scaled by 1/sqrt(T);
  logits = [[S01, S00\diag], [S10, S11\diag]]  (2BN x 2BN-1)
  loss = mean_i(logsumexp(row_i) - label_i),  label_i = S01[i,i].

Per-core plan (data-parallel rows; host np.roll makes the program
core-independent; each core's rows are cols 0..511 of each view):
  setup(r):  8x 1MB DMA raw [E, BN] f32 (sync queue); squares (DVE +
    gpsimd, bf16); per-column sumsq via ones-matmul (bf16, 1 cyc/row)
    -> [1,2048] PSUM; tiny reshape-DMA -> [8,512] SBUF; ACT Ln+Exp
    (one table set) -> scl; one-hot-row broadcast matmuls (f32r) ->
    pb [128,2048] PSUM; DVE raw*pb -> bf16 nrm.
  main(r):   per 128-row block x 2048-key tile: 8 bf16 matmuls K=256
    -> PSUM; ACT exp with accum_out row-sums (32 tiles = the ACT
    bottleneck, ~2.04us each).
  epilogue:  row-sums - exp(10) (same-view diag is exactly 10, so no
    diag extraction); ln via ACT with bias=-exp(10), accum_out;
    partition-reduce via ones-matmul; labels = colsum(nrm0*nrm1).
Reps are software-pipelined: setup(r) emission is interleaved into
main(r-1) so every engine queue stays busy; steady state is ACT-bound.
Host sums the 8 per-core partials and divides by 2*B*N.
"""

import math
from contextlib import ExitStack

import numpy as np

import concourse.bacc as bacc
import concourse.bass as bass
import concourse.tile as tile
from concourse import mybir
from concourse.bass_utils import run_bass_kernel_spmd

B, E, N = 64, 256, 64
BN = B * N            # 4096 rows per view
NCORES = 8
P = 128
KH = E // P           # 2 contraction halves
G = 2048              # column group width (setup granularity)
PW = 2048             # PSUM tile free width
NJH = G // PW         # PSUM tiles per column group
PSUM_BUFS = 16384 // (PW * 4)
NG = BN // G          # 2 column groups
GB = B // NG          # b-range per column group
TEMP = 0.1
SCALE_BIAS = -0.5 * math.log(TEMP)   # exp(-0.5*ln(s) + BIAS) = sqrt(10/s)
EXP10 = float(np.exp(np.float64(10.0)))  # exact same-view diag: |q|^2 = 10

F32 = mybir.dt.float32
F32R = mybir.dt.float32r
BF16 = mybir.dt.bfloat16
FP8 = mybir.dt.float8e4
AFT = mybir.ActivationFunctionType

# We alternate Ln and Exp on the ACT engine every rep. The table-load
# inserter picks the first set containing each function, which puts Ln and
# Exp in different sets and forces a ~1.3us ACT table reload per switch.
# Hide Exp/Ln from every set except the one that contains both, so all
# activations share one resident table (json set indices are preserved).
_orig_gat = bacc.get_activation_tables


def _gat_ln_exp_combined(arch):
    tabs = {k: set(v) for k, v in _orig_gat(arch).items()}
    for name, s in tabs.items():
        if name != "natural_log_exp_and_others":
            s.discard(AFT.Exp)
            s.discard(AFT.Ln)
    return tabs


bacc.get_activation_tables = _gat_ln_exp_combined


def _main_tile_list():
    """(g, half, m, tg) in emission order: 16 g0 tiles then 16 g1 tiles."""
    out = []
    for g in range(NG):
        for half in range(2):
            for m in range(4):
                for tg in range(2):
                    out.append((g, half, m, tg))
    return out


class _Emitter:
    def __init__(self, nc, pl):
        self.nc = nc
        self.pl = pl
        self.ones_col = None
        self.ones_col_b = None
        self.onesel = None
        # per-rep state
        self.raw = {}     # r -> [v][h] tiles
        self.sq = {}      # (r, g) -> {(v, h): tile}
        self.nrm = {}     # r -> [v][h] tiles
        self.scl = {}     # (r, g) -> scl16 tile
        self.stats = {}   # r -> stats tile
        self.lbl2 = {}    # r -> 2*sum(labels) tile

    def emit_consts(self):
        nc, pl = self.nc, self.pl
        self.ones_col = pl["cst"].tile([P, 1], F32, tag="ones_col",
                                       name="ones_col")
        nc.vector.memset(self.ones_col[:], 1.0)
        self.ones_col_b = pl["cst"].tile([P, 1], BF16, tag="ones_col_b",
                                         name="ones_col_b")
        nc.vector.memset(self.ones_col_b[:], 1.0)
        self.ones_row = pl["cst"].tile([1, P], BF16, tag="ones_row",
                                       name="ones_row")
        nc.vector.memset(self.ones_row[:], 1.0)
        self.sbias = pl["cst"].tile([8, 1], F32, tag="sbias", name="sbias")
        nc.vector.memset(self.sbias[:], SCALE_BIAS)
        self.nexp10 = pl["cst"].tile([P, 1], F32, tag="nexp10", name="nexp10")
        nc.vector.memset(self.nexp10[:], -EXP10)
        self.zbias = pl["cst"].tile([8, 1], F32, tag="zbias", name="zbias")
        nc.vector.memset(self.zbias[:], 0.0)

    # ---- setup pieces -------------------------------------------------
    def setup_dma_and_sq_g0(self, r, vin):
        """Raw loads for the whole rep + squares for g0 (and gpsimd g1)."""
        nc, pl = self.nc, self.pl
        raw = [[pl["raw"].tile([P, BN], F32, tag=f"raw{v}{h}",
                               name=f"raw{v}{h}_{r}")
                for h in range(KH)] for v in range(2)]
        self.raw[r] = raw
        for g in range(NG):
            for v in range(2):
                for h in range(KH):
                    src = vin[v][g * GB:(g + 1) * GB, h * P:(h + 1) * P, :] \
                        .rearrange("b e n -> e b n")
                    dst = raw[v][h][:, g * G:(g + 1) * G].rearrange(
                        "e (b n) -> e b n", b=GB)
                    nc.sync.dma_start(out=dst, in_=src)
        # fp8 DoubleRow layout: [K=128, k-subtile, col] per view
        self.nrm[r] = [pl["nrm"].tile([P, KH, BN], FP8, tag=f"nrm{v}",
                                      name=f"nrm{v}_{r}")
                       for v in range(2)]
        # squares: h==0 on DVE, h==1 on gpsimd; gpsimd also takes g1 now
        self._emit_sq(r, 0, engines=("vector", "gpsimd"))
        self._emit_sq(r, 1, engines=(None, "gpsimd"))

    def _emit_sq(self, r, g, engines):
        nc, pl = self.nc, self.pl
        d = self.sq.setdefault((r, g), {})
        gs = slice(g * G, (g + 1) * G)
        for v in range(2):
            for h in range(KH):
                eng = engines[h]
                if eng is None or (v, h) in d:
                    continue
                t = pl["sq"].tile([P, G], BF16, tag="sq", name=f"sq{v}{h}{g}_{r}")
                getattr(nc, eng).tensor_mul(
                    t[:], self.raw[r][v][h][:, gs], self.raw[r][v][h][:, gs])
                d[(v, h)] = t

    def setup_colsum_g(self, r, g):
        """Per-column sumsq matmuls -> PSUM row -> SBUF [8,512] stage."""
        nc, pl = self.nc, self.pl
        if g == 0:
            self._emit_sq(r, 1, engines=("vector", None))
        sq = self.sq[(r, g)]
        sstg = pl["stg"].tile([8, 512], F32, tag="sstg", name=f"sstg{g}_{r}")
        for v in range(2):
            sres = pl["stg"].tile([1, G], F32, tag="sres",
                                  name=f"sres{v}{g}_{r}")
            for bh in range(NJH):
                ss = pl["psum"].tile([P, PW], F32, tag="ps",
                                     name=f"ss{v}{g}{bh}_{r}")
                for b in range(PW // 512):
                    js = slice(b * 512, (b + 1) * 512)
                    for h in range(KH):
                        nc.tensor.matmul(
                            ss[0:1, js], self.ones_col_b[:],
                            sq[(v, h)][:, bh * PW + b * 512:
                                       bh * PW + (b + 1) * 512],
                            start=(h == 0), stop=(h == KH - 1))
                # PSUM -> SBUF bounce (DMA can't read PSUM)
                nc.vector.tensor_copy(
                    sres[0:1, bh * PW:(bh + 1) * PW], ss[0:1, :])
            # reshape [1,2048] -> [4,512] rows so Ln/Exp use 8 ACT lanes
            nc.gpsimd.dma_start(out=sstg[v * 4:(v + 1) * 4, :], in_=sres[:])
        self.scl[(r, g, "sstg")] = sstg

    def setup_scale_g(self, r, g):
        """Ln/Exp -> stride-0 broadcast DMA into SBUF pb -> apply."""
        nc, pl = self.nc, self.pl
        sstg = self.scl.pop((r, g, "sstg"))
        gs = slice(g * G, (g + 1) * G)
        lnstg = pl["stg"].tile([8, 512], F32, tag="lnstg", name=f"ln{g}_{r}")
        nc.scalar.activation(lnstg[:], sstg[:], AFT.Ln, bias=self.zbias[:])
        scl16 = pl["stg"].tile([8, 512], BF16, tag="scl16", name=f"scl{g}_{r}")
        nc.scalar.activation(scl16[:], lnstg[:], AFT.Exp,
                             scale=-0.5, bias=self.sbias[:])
        self.scl[(r, g)] = scl16
        # broadcast each scale row across 128 partitions via DMA (keeps the
        # PE free of any dependency on the ACT-produced scales)
        for v in range(2):
            srow = pl["stg"].tile([1, G], BF16, tag="srow",
                                  name=f"srow{v}{g}_{r}")
            nc.gpsimd.dma_start(out=srow[:], in_=scl16[v * 4:(v + 1) * 4, :])
            pb = pl["pbs"].tile([P, G], BF16, tag=f"pb{v}",
                                name=f"pb{v}{g}_{r}")
            nc.gpsimd.partition_broadcast(pb[:], srow[0:1, :])
            for h in range(KH):
                nc.vector.tensor_mul(
                    self.nrm[r][v][:, h, gs], self.raw[r][v][h][:, gs], pb[:])

    def emit_label(self, r):
        """2 * sum_i(label_i) for this core's 512 rows (bf16 path)."""
        nc, pl = self.nc, self.pl
        nrm = self.nrm[r]
        tmps = []
        for h in range(KH):
            t = pl["sml"].tile([P, 512], BF16, tag=f"lblt{h}",
                               name=f"lblt{h}_{r}")
            nc.vector.tensor_mul(t[:], nrm[0][:, h, 0:512],
                                 nrm[1][:, h, 0:512])
            tmps.append(t)
        lbl = pl["psum"].tile([P, PW], F32, tag="ps", name=f"lbl_{r}")
        for h in range(KH):
            nc.tensor.matmul(lbl[0:1, 0:512], self.ones_col_b[:], tmps[h][:],
                             start=(h == 0), stop=(h == KH - 1))
        lbls = pl["sml"].tile([1, 1], F32, tag="lbls", name=f"lbls_{r}")
        nc.vector.tensor_reduce(lbls[:], lbl[0:1, 0:512],
                                axis=mybir.AxisListType.X,
                                op=mybir.AluOpType.add)
        lbl2 = pl["sml"].tile([1, 1], F32, tag="lbl2", name=f"lbl2_{r}")
        nc.vector.tensor_scalar_mul(lbl2[:], lbls[:], 2.0)
        self.lbl2[r] = lbl2

    # ---- main pieces --------------------------------------------------
    def main_tiles(self, r, tiles):
        nc, pl = self.nc, self.pl
        nrm = self.nrm[r]
        if r not in self.stats:
            self.stats[r] = pl["sml"].tile([P, 32 * NJH], F32, tag="stats",
                                           name=f"stats_{r}")
        stats = self.stats[r]
        for (g, half, m, tg) in tiles:
            ms = slice(m * P, (m + 1) * P)
            keys = nrm[1 - half] if tg == 0 else nrm[half]
            for jh in range(NJH):
                goff = g * G + jh * PW
                pt = pl["psum"].tile([P, PW], F32, tag="ps",
                                     name=f"pt{g}{half}{m}{tg}{jh}_{r}")
                for j in range(PW // 512):
                    js = slice(j * 512, (j + 1) * 512)
                    nc.tensor.matmul(
                        pt[:, js], nrm[half][:, :, ms],
                        keys[:, :, goff + j * 512: goff + (j + 1) * 512],
                        perf_mode=mybir.MatmulPerfMode.DoubleRow)
                esc = pl["esc"].tile([P, PW], BF16, tag="esc",
                                     name=f"esc{g}{half}{m}{tg}{jh}_{r}")
                sidx = (((half * 4 + m) * 2 + tg) * 2 + g) * NJH + jh
                nc.scalar.activation(esc[:], pt[:, :], AFT.Exp,
                                     accum_out=stats[:, sidx:sidx + 1])

    def emit_epilogue(self, r, out_dram):
        nc, pl = self.nc, self.pl
        stats = self.stats[r]
        rows = pl["sml"].tile([P, 8], F32, tag="rows", name=f"rows_{r}")
        nc.vector.tensor_reduce(
            rows[:], stats[:].rearrange("p (m t) -> p m t", t=4 * NJH),
            axis=mybir.AxisListType.X, op=mybir.AluOpType.add)
        lnr = pl["sml"].tile([P, 8], F32, tag="lnr", name=f"lnr_{r}")
        lnsum = pl["sml"].tile([P, 1], F32, tag="lnsum", name=f"lnsum_{r}")
        # ln(rowsum - exp(10)): removes the same-view diag term exactly
        nc.scalar.activation(lnr[:], rows[:], AFT.Ln, bias=self.nexp10[:],
                             accum_out=lnsum[:])
        fp = pl["psum"].tile([P, PW], F32, tag="ps", name=f"fp_{r}")
        nc.tensor.matmul(fp[0:1, 0:1], lnsum[:], self.ones_col[:])
        res = pl["sml"].tile([1, 1], F32, tag="res", name=f"res_{r}")
        nc.vector.tensor_sub(res[:], fp[0:1, 0:1], self.lbl2[r][:])
        nc.gpsimd.dma_start(out=out_dram[:], in_=res[:])
        # free per-rep references
        for d in (self.raw, self.nrm, self.stats, self.lbl2):
            d.pop(r, None)
        for g in range(NG):
            self.sq.pop((r, g), None)
            self.scl.pop((r, g), None)


def _build_nc(reps: int = 1, mode: str = "full"):
    nc = bacc.Bacc()
    vin = [
        nc.dram_tensor("view0", [B, E, N], F32, kind="ExternalInput"),
        nc.dram_tensor("view1", [B, E, N], F32, kind="ExternalInput"),
    ]
    out_dram = nc.dram_tensor("out", [1, 1], F32, kind="ExternalOutput")

    with ExitStack() as ctx:
        tc = ctx.enter_context(tile.TileContext(nc))
        pl = {
            "raw": ctx.enter_context(tc.tile_pool(name="raw", bufs=1)),
            "nrm": ctx.enter_context(tc.tile_pool(name="nrm", bufs=2)),
            "sq": ctx.enter_context(tc.tile_pool(name="sq", bufs=5)),
            "esc": ctx.enter_context(tc.tile_pool(name="esc", bufs=1)),
            "pbs": ctx.enter_context(tc.tile_pool(name="pbs", bufs=2)),
            "stg": ctx.enter_context(tc.tile_pool(name="stg", bufs=2)),
            "sml": ctx.enter_context(tc.tile_pool(name="sml", bufs=2)),
            "cst": ctx.enter_context(tc.tile_pool(name="cst", bufs=1)),
            "psum": ctx.enter_context(
                tc.tile_pool(name="psum", bufs=PSUM_BUFS, space="PSUM")),
        }
        em = _Emitter(nc, pl)
        em.emit_consts()
        tiles = _main_tile_list()
        if mode == "main":
            # one setup, then reps x main phase (timing attribution)
            em.setup_dma_and_sq_g0(0, vin)
            for g in range(NG):
                em.setup_colsum_g(0, g)
                em.setup_scale_g(0, g)
            em.emit_label(0)
            nrm0, lbl20 = em.nrm[0], em.lbl2[0]
            for r in range(reps):
                em.nrm[r], em.lbl2[r] = nrm0, lbl20
                em.main_tiles(r, tiles)
                em.emit_epilogue(r, out_dram)
                em.nrm[0], em.lbl2[0] = nrm0, lbl20
            nc.compile()
            return nc
        if mode == "setup":
            for r in range(reps):
                em.setup_dma_and_sq_g0(r, vin)
                for g in range(NG):
                    em.setup_colsum_g(r, g)
                    em.setup_scale_g(r, g)
                em.emit_label(r)
                del em.raw[r], em.nrm[r]
            nc.compile()
            return nc
        for r in range(reps):
            em.setup_dma_and_sq_g0(r, vin)
            if r > 0:
                em.main_tiles(r - 1, tiles[0:12])
            em.setup_colsum_g(r, 0)
            if r > 0:
                em.main_tiles(r - 1, tiles[12:16])
            em.setup_scale_g(r, 0)
            if r > 0:
                em.main_tiles(r - 1, tiles[16:24])
            em.setup_colsum_g(r, 1)
            if r > 0:
                em.main_tiles(r - 1, tiles[24:26])
            em.setup_scale_g(r, 1)
            if r > 0:
                em.main_tiles(r - 1, tiles[26:32])
            em.emit_label(r)
            if r > 0:
                em.emit_epilogue(r - 1, out_dram)
        em.main_tiles(reps - 1, tiles)
        em.emit_epilogue(reps - 1, out_dram)

    nc.compile()
    return nc


_NC_CACHE = None


def _run_spmd(view0: np.ndarray, view1: np.ndarray, nc=None, **spmd_kwargs):
    global _NC_CACHE
    if nc is None:
        if _NC_CACHE is None:
            _NC_CACHE = _build_nc()
        nc = _NC_CACHE

    in_maps = []
    for c in range(NCORES):
        in_maps.append({
            "view0": np.ascontiguousarray(
                np.roll(view0, -c * (B // NCORES), axis=0)),
            "view1": np.ascontiguousarray(
                np.roll(view1, -c * (B // NCORES), axis=0)),
        })
    res = run_bass_kernel_spmd(nc, in_maps, core_ids=list(range(NCORES)),
                               **spmd_kwargs)
    total = sum(float(r["out"][0, 0]) for r in res.results)
    return np.float32(total / (2 * BN)), res


def kernel(view0: np.ndarray, view1: np.ndarray) -> np.ndarray:
    loss, _ = _run_spmd(view0, view1)
    return loss
